# revision 19
# baseline (speedup 1.0000x reference)
"""Malvar-He-Cutler demosaic on 8 Trainium2 NeuronCores.

kernel(**inputs) takes the FULL inputs (x int32 (4096,6144), kernels
(4,1,5,5) fp32) and returns the FULL (4096,6144,3) int32 output.

Row sharding: each core gets a 512-row band (reflect padding, dtype
conversion and output assembly are host-side).

Device computes only the 8 CONV (plane-set x row-parity) combos; the 4
raw-passthrough combos are filled host-side from x directly. The 12
(channel x row-parity x col-parity) output slots pair up into 4 conv
plane-sets, each a banded matmul accumulation over the horizontal taps:

    P0 GR@even-cols : even rows G_at_RB   (->G), odd rows R_at_G_Brow (->R)
    P1 B @even-cols : even rows R_at_B    (->B), odd rows R_at_G_Rrow (->B)
    P2 R @odd-cols  : even rows R_at_G_Rrow(->R), odd rows R_at_B     (->R)
    P3 BG@odd-cols  : even rows R_at_G_Brow(->B), odd rows G_at_RB    (->G)

Banded lhsT matrices encode vertical taps AND per-row-parity kernel
selection.

Fast path (fp8, used when the kernel taps and taps/32 are exactly
representable in e4m3 -- true for the MHC constants): 4 row-chunks of
128, contraction K=132 folded into 2 k-tiles of 66 partitions, fp8
DoubleRow matmuls at 2 rows/cycle. Input is sent twice: hi = e4m3 of
x*2^-17 and lo = e4m3 of the residual (x - hi*2^17)*2^-12. Each
(plane, cc) accumulation runs 5 hi streams (dx 0..4) plus one lo
stream at dx=2 with weights/32 -- the center column carries most of
the weight mass, so this residual cuts the fp8 quantization error from
~1.8e-2 to ~1.0e-2 for 20% more matmuls.

Fallback path (fp16, arbitrary kernels): 5 row-chunks of <=124 rows,
input scaled by 2^-10, 5 dx streams per (plane, cc).

PSUM evictions (fp32->fp16 cast) split across Vector and Activation
engines; contiguous per-(chunk, plane) DMAs on the gpsimd swdge queue
store fp16 planes to HBM; host rescales, fills passthrough, clips.

_split_waits post-pass: this container's walrus accepts only ONE
semaphore wait per instruction, so excess Tile-emitted waits are
hoisted onto preceding same-engine NOPs (sequencer order preserves
semantics).
"""

import sys

import numpy as np

sys.path.insert(0, "/opt/trn_rl_repo")

H, W = 4096, 6144
NCORES = 8
RB = H // NCORES          # 512 output rows per core
CW = 1024                 # output columns per column-chunk
NPS = 4                   # conv plane-sets
NCC = W // CW             # 6 column-chunks
NCOL = CW // 2            # 512 outputs per column parity per cc
PW = NCC * NCOL           # 3072: fp16 plane width (one column parity)

# fp16 fallback path
CH16 = 124                # max output rows per chunk
XSCALE16 = 2.0 ** -10     # input prescale so raw values fit fp16

# fp8 fast path
CH8 = 128                 # output rows per chunk (4 chunks of 128)
KT8 = 66                  # k-tile partitions (132 = 2*66 contraction rows)
NST8 = 6                  # streams per accumulation: 5 hi dx + 1 lo center
HISC = 2.0 ** -17         # hi prescale: x*HISC < 128 fits e4m3
LOSC = 2.0 ** -12         # lo prescale: |residual|*LOSC <= 128


def _psdef(kernels: np.ndarray):
    K = kernels[:, 0].astype(np.float32)
    return [
        (K[0], K[2]),  # P0 GR@e: even rows G_at_RB,     odd rows R_at_G_Brow
        (K[3], K[1]),  # P1 B @e: even rows R_at_B,      odd rows R_at_G_Rrow
        (K[1], K[3]),  # P2 R @o: even rows R_at_G_Rrow, odd rows R_at_B
        (K[2], K[0]),  # P3 BG@o: even rows R_at_G_Brow, odd rows G_at_RB
    ]


def _fp8_exact(kernels: np.ndarray) -> bool:
    import ml_dtypes

    K = kernels[:, 0].astype(np.float32)
    for scale in (1.0, 1.0 / 32.0):
        v = K * np.float32(scale)
        q = v.astype(ml_dtypes.float8_e4m3).astype(np.float32)
        if not np.array_equal(q, v):
            return False
    return True


def _row_chunks16():
    # small first chunk: its input load is quick, so compute starts early
    # (matmul cost depends only on the moving dim, not on chunk rows)
    sizes = [24, 122, 122, 122, 122]
    out, r0 = [], 0
    for s in sizes:
        out.append((r0, s))
        r0 += s
    assert r0 == RB
    return out


def _build_weights16(kernels: np.ndarray) -> np.ndarray:
    """Banded lhsT matrices, shape (128, NPS*5*CH16) fp16, k-major."""
    wts = np.zeros((128, NPS * 5, CH16), np.float32)
    p = np.arange(CH16)
    for ps, (ke, ko) in enumerate(_psdef(kernels)):
        for dxi in range(5):
            i = ps * 5 + dxi
            for dyi in range(5):
                k = p + dyi
                ok = k < 128
                w = np.where(p % 2 == 0, ke[dyi, dxi], ko[dyi, dxi])
                wts[k[ok], i, p[ok]] = w[ok]
    return wts.reshape(128, NPS * 5 * CH16).astype(np.float16)


def _build_weights8(kernels: np.ndarray) -> np.ndarray:
    """DoubleRow banded lhsT, shape (KT8, 2*NPS*NST8*CH8) e4m3.

    Free layout per (ps, stream) block: [t, m] with t the k-tile.
    Stream 0..4 = hi taps dx 0..4; stream 5 = center taps / 32 on lo.
    """
    import ml_dtypes

    wts = np.zeros((KT8, 2, NPS * NST8, CH8), np.float32)
    m = np.arange(CH8)
    for ps, (ke, ko) in enumerate(_psdef(kernels)):
        for s in range(NST8):
            dxi = s if s < 5 else 2
            scale = 1.0 if s < 5 else 1.0 / 32.0
            i = ps * NST8 + s
            for dyi in range(5):
                k = m + dyi          # contraction row, in [0, 132)
                t = k // KT8
                p = k % KT8
                w = np.where(m % 2 == 0, ke[dyi, dxi], ko[dyi, dxi]) * scale
                wts[p, t, i, m] = w
    wts = wts.transpose(0, 2, 1, 3)  # (p, i, t, m): per-block [t, m] layout
    return np.ascontiguousarray(wts).astype(ml_dtypes.float8_e4m3)


def _split_waits(nc, maxw=1):
    """Hoist excess semaphore waits onto preceding same-engine NOPs."""
    import concourse.mybir as mybir

    nsplit = 0
    for f in nc.m.functions:
        for b in f.blocks:
            new = []
            for inst in list(b.instructions):
                si = inst.sync_info
                ow = list(si.on_wait) if si and si.on_wait else []
                if len(ow) > maxw:
                    for wx in ow[:-maxw]:
                        new.append(mybir.InstNoOp(
                            name=inst.name + f"-w{nsplit}",
                            sync_info=mybir.SyncInfo(on_wait=[wx], on_update=[]),
                            engine=inst.engine,
                            bass_nofuse=True,
                        ))
                        nsplit += 1
                    si.on_wait = ow[-maxw:]
                new.append(inst)
            b.instructions = new
    return nsplit


def _build_bass16():
    import contextlib

    import concourse.bass as bass
    import concourse.mybir as mybir
    import concourse.tile as tile

    f32 = mybir.dt.float32
    f16 = mybir.dt.float16

    rowchunks = _row_chunks16()

    nc = bass.Bass()
    xb = nc.declare_dram_parameter("xb", [RB + 4, W + 4], f16, isOutput=False)
    wts = nc.declare_dram_parameter("wts", [128, NPS * 5 * CH16], f16,
                                    isOutput=False)
    out = nc.declare_dram_parameter("out", [NPS * RB, PW], f16, isOutput=True)

    with contextlib.ExitStack() as ctx:
        tc = ctx.enter_context(tile.TileContext(nc))
        wpool = ctx.enter_context(tc.tile_pool(name="wpool", bufs=1))
        inpool = ctx.enter_context(tc.tile_pool(name="inpool", bufs=1))
        opool = ctx.enter_context(tc.tile_pool(name="opool", bufs=2))
        pspool = ctx.enter_context(tc.tile_pool(name="pspool", bufs=2,
                                                space="PSUM"))

        wtile = wpool.tile([128, NPS * 5 * CH16], f16)
        nc.scalar.dma_start(wtile[:], wts[:])

        itiles = []
        for g, (r0, rows) in enumerate(rowchunks):
            it = inpool.tile([128, W + 4], f16, tag=f"it{g}", name=f"it{g}")
            eng = nc.sync if g % 2 == 0 else nc.scalar
            eng.dma_start(it[: rows + 4, :], xb[r0 : r0 + rows + 4, :])
            itiles.append(it)

        for g, (r0, rows) in enumerate(rowchunks):
            krows = rows + 4
            otiles = [
                opool.tile([128, PW], f16, tag=f"ot{ps}", name=f"ot{ps}g{g}")
                for ps in range(NPS)
            ]
            last = g == len(rowchunks) - 1
            for ps in range(NPS):
                colpar = 0 if ps < 2 else 1
                for cc in range(NCC):
                    ptile = pspool.tile([128, 512], f32, tag=f"b{cc % 4}",
                                        name=f"pt{g}_{ps}_{cc}")
                    for dxi in range(5):
                        i = ps * 5 + dxi
                        lhsT = wtile[:krows, i * CH16 : i * CH16 + rows]
                        c0 = CW * cc + colpar + dxi
                        rhs = itiles[g][:krows, c0 : c0 + 2 * NCOL - 1 : 2]
                        nc.tensor.matmul(
                            ptile[:rows, :NCOL],
                            lhsT,
                            rhs,
                            start=(dxi == 0),
                            stop=(dxi == 4),
                        )
                    dst = otiles[ps][:rows, cc * NCOL : (cc + 1) * NCOL]
                    if cc % 2 == 0:
                        nc.vector.tensor_copy(dst, ptile[:rows, :NCOL])
                    else:
                        nc.scalar.copy(dst, ptile[:rows, :NCOL])
                    if last and cc % 3 == 2:
                        nc.gpsimd.dma_start(
                            out[ps * RB + r0 : ps * RB + r0 + rows,
                                (cc - 2) * NCOL : (cc + 1) * NCOL],
                            otiles[ps][:rows, (cc - 2) * NCOL : (cc + 1) * NCOL],
                        )
                if not last:
                    nc.gpsimd.dma_start(
                        out[ps * RB + r0 : ps * RB + r0 + rows, :],
                        otiles[ps][:rows, :],
                    )
    _split_waits(nc)
    return nc


def _build_bass8():
    import contextlib

    import concourse.bass as bass
    import concourse.mybir as mybir
    import concourse.tile as tile

    f32 = mybir.dt.float32
    f16 = mybir.dt.float16
    f8 = mybir.dt.float8e4

    ngc = RB // CH8  # 4 chunks of 128 rows

    nc = bass.Bass()
    xhi = nc.declare_dram_parameter("xhi", [ngc * KT8, 2, W + 4], f8,
                                    isOutput=False)
    xlo = nc.declare_dram_parameter("xlo", [ngc * KT8, 2, W + 4], f8,
                                    isOutput=False)
    wts = nc.declare_dram_parameter("wts", [KT8, NPS * NST8, 2, CH8], f8,
                                    isOutput=False)
    out = nc.declare_dram_parameter("out", [NPS * RB, PW], f16, isOutput=True)

    with contextlib.ExitStack() as ctx:
        tc = ctx.enter_context(tile.TileContext(nc))
        wpool = ctx.enter_context(tc.tile_pool(name="wpool", bufs=1))
        inpool = ctx.enter_context(tc.tile_pool(name="inpool", bufs=1))
        opool = ctx.enter_context(tc.tile_pool(name="opool", bufs=2))
        pspool = ctx.enter_context(tc.tile_pool(name="pspool", bufs=2,
                                                space="PSUM"))

        wtile = wpool.tile([128, NPS * NST8, 2, CH8], f8)
        nc.scalar.dma_start(wtile[:KT8], wts[:])

        # interleave hi/lo chunk loads across the two hwdge queues
        this_, tlos = [], []
        for g in range(ngc):
            th = inpool.tile([128, 2, W + 4], f8, tag=f"th{g}", name=f"th{g}")
            tl = inpool.tile([128, 2, W + 4], f8, tag=f"tl{g}", name=f"tl{g}")
            ehi = nc.sync if g % 2 == 0 else nc.scalar
            elo = nc.scalar if g % 2 == 0 else nc.sync
            ehi.dma_start(th[:KT8], xhi[g * KT8 : (g + 1) * KT8])
            elo.dma_start(tl[:KT8], xlo[g * KT8 : (g + 1) * KT8])
            this_.append(th)
            tlos.append(tl)

        dr = mybir.MatmulPerfMode.DoubleRow
        for g in range(ngc):
            r0 = g * CH8
            otiles = [
                opool.tile([128, PW], f16, tag=f"ot{ps}", name=f"ot{ps}g{g}")
                for ps in range(NPS)
            ]
            last = g == ngc - 1
            for ps in range(NPS):
                colpar = 0 if ps < 2 else 1
                for cc in range(NCC):
                    ptile = pspool.tile([128, 512], f32, tag=f"b{cc % 4}",
                                        name=f"pt{g}_{ps}_{cc}")
                    for s in range(NST8):
                        dxi = s if s < 5 else 2
                        src = this_[g] if s < 5 else tlos[g]
                        i = ps * NST8 + s
                        c0 = CW * cc + colpar + dxi
                        rhs = src[:KT8, :, c0 : c0 + 2 * NCOL - 1 : 2]
                        nc.tensor.matmul(
                            ptile[:CH8, :NCOL],
                            wtile[:KT8, i],
                            rhs,
                            start=(s == 0),
                            stop=(s == NST8 - 1),
                            perf_mode=dr,
                        )
                    dst = otiles[ps][:CH8, cc * NCOL : (cc + 1) * NCOL]
                    if cc % 2 == 0:
                        nc.vector.tensor_copy(dst, ptile[:CH8, :NCOL])
                    else:
                        nc.scalar.copy(dst, ptile[:CH8, :NCOL])
                    if last and cc % 3 == 2:
                        nc.gpsimd.dma_start(
                            out[ps * RB + r0 : ps * RB + r0 + CH8,
                                (cc - 2) * NCOL : (cc + 1) * NCOL],
                            otiles[ps][:CH8, (cc - 2) * NCOL : (cc + 1) * NCOL],
                        )
                if not last:
                    nc.gpsimd.dma_start(
                        out[ps * RB + r0 : ps * RB + r0 + CH8, :],
                        otiles[ps][:CH8, :],
                    )
    _split_waits(nc)
    return nc


_BASS_CACHE = {}


def _get_nc(mode):
    if mode not in _BASS_CACHE:
        _BASS_CACHE[mode] = _build_bass8() if mode == "fp8" else _build_bass16()
    return _BASS_CACHE[mode]


def _fold8(a: np.ndarray) -> np.ndarray:
    """(516, W+4) -> (ngc*KT8, 2, W+4): per chunk [p, t, c] k-tile fold."""
    ngc = RB // CH8
    blocks = []
    for g in range(ngc):
        b = a[g * CH8 : g * CH8 + 2 * KT8]            # (132, W+4)
        blocks.append(b.reshape(2, KT8, W + 4).transpose(1, 0, 2))
    return np.ascontiguousarray(np.concatenate(blocks, axis=0))


def _prep_inputs(x: np.ndarray, kernels: np.ndarray, mode: str):
    xp = np.pad(x, 2, mode="reflect").astype(np.float32)
    in_maps = []
    if mode == "fp8":
        import ml_dtypes

        f8 = ml_dtypes.float8_e4m3
        hi8 = (xp * np.float32(HISC)).astype(f8)
        lo8 = ((xp - hi8.astype(np.float32) * np.float32(1.0 / HISC))
               * np.float32(LOSC)).astype(f8)
        wts = _build_weights8(kernels)
        for c in range(NCORES):
            sl = slice(c * RB, c * RB + RB + 4)
            in_maps.append({
                "xhi": _fold8(hi8[sl]),
                "xlo": _fold8(lo8[sl]),
                "wts": wts,
            })
    else:
        xp16 = (xp * np.float32(XSCALE16)).astype(np.float16)
        wts = _build_weights16(kernels)
        for c in range(NCORES):
            band = np.ascontiguousarray(xp16[c * RB : c * RB + RB + 4, :])
            in_maps.append({"xb": band, "wts": wts})
    return in_maps


def _assemble(x: np.ndarray, parts: list, mode: str) -> np.ndarray:
    """Host-side assembly: scale conv planes, fill passthrough, clip."""
    upscale = np.float32(1.0 / HISC if mode == "fp8" else 1.0 / XSCALE16)
    planes = [
        np.concatenate([p[ps * RB : (ps + 1) * RB] for p in parts], axis=0)
        .astype(np.float32) * upscale
        for ps in range(NPS)
    ]
    p0, p1, p2, p3 = planes
    outf = np.empty((H, W, 3), np.float32)
    outf[0::2, 0::2, 1] = p0[0::2]   # G at (even r, even c)
    outf[1::2, 0::2, 0] = p0[1::2]   # R at (odd r, even c)
    outf[:, 0::2, 2] = p1            # B at even cols
    outf[:, 1::2, 0] = p2            # R at odd cols
    outf[0::2, 1::2, 2] = p3[0::2]   # B at (even r, odd c)
    outf[1::2, 1::2, 1] = p3[1::2]   # G at (odd r, odd c)
    # raw passthrough (exact int values, within [0, 2^24))
    outf[0::2, 0::2, 0] = x[0::2, 0::2]
    outf[0::2, 1::2, 1] = x[0::2, 1::2]
    outf[1::2, 0::2, 1] = x[1::2, 0::2]
    outf[1::2, 1::2, 2] = x[1::2, 1::2]
    np.clip(outf, 0.0, 16777215.0, out=outf)
    return outf.astype(np.int32)


def kernel(x: np.ndarray, kernels: np.ndarray) -> np.ndarray:
    from concourse.bass_utils import run_bass_kernel_spmd

    x = np.asarray(x)
    kernels = np.asarray(kernels)
    assert x.shape == (H, W) and x.dtype == np.int32

    mode = "fp8" if _fp8_exact(kernels) else "fp16"
    in_maps = _prep_inputs(x, kernels, mode)
    nc = _get_nc(mode)
    res = run_bass_kernel_spmd(nc, in_maps, core_ids=list(range(NCORES)))
    parts = [res.results[c]["out"] for c in range(NCORES)]
    return _assemble(x, parts, mode)
